# revision 5
# baseline (speedup 1.0000x reference)
"""Bass/Trainium2 kernel for a 2-layer bidirectional QRNN (fo-pooling).

Reference computation (per layer, per direction):
    ZFO = X @ W + b            # [S, B, 3H]
    Z, F, O = split(ZFO); Z = tanh(Z); F = sigmoid(F); O = sigmoid(O)
    c_t = F_t * c_{t-1} + (1 - F_t) * Z_t        (bw direction: reversed time)
    Y_dir = O * C
    Y = concat(Y_fw, Y_bw)     # [S, B, 2H]
Two stacked layers; output is [B, S, 2H].

Sharding: data-parallel over batch. B=16 rows -> 2 rows per NeuronCore x 8.
Each core runs both layers for its 2 rows; no collectives.

v2 design (vs the fp32r/DRAM-y1 v1 baseline, 635us):
- bf16 matmuls. Measured 216ns/[128x128x512] matmul vs 233ns fp32r (the
  fp32r moving operand saturates SBUF read bandwidth); LDWEIGHTS halves.
  End-to-end rel err ~4e-3 (gate is 2e-2).
- The inter-layer activation Y1 lives entirely in SBUF as bf16 (8 chunk
  tiles [128, S] per row), never touching DRAM. Rows are processed
  b-major (L0fw, L0bw, L1fw, L1bw per row) so only one row's Y1 is live.
- Z-gate weight/bias columns are negated on the host, so the Tanh
  activation directly yields zneg = -z and the scan's g-input
  g = (1-f)*z = (f-1)*zneg is ONE fused scalar_tensor_tensor op.
- Engine balance in layer 0 (PE: 10.4us/s-tile): Scalar = 3 activations
  per h-chunk; DVE = scan + y-mult; GpSimd = input casts + g + carry
  columns. Input DMA+cast for s-tile i+1 issue at the START of s-tile i
  so the PE never starves at iteration boundaries.
- Weight loads: w0f is loaded eagerly, split across the two HW DGE
  queues; the other three weight sets stream in as small drip items
  (double-chunk DMAs + per-chunk casts) through row 0's iterations.

The time recurrence uses the DVE `tensor_tensor_scan` instruction
(state = f*state + g along the free axis); the bw direction runs the scan
through reversed access patterns with s-tiles processed in descending
order, chaining the carry via a [128,1] column copy.
"""

import numpy as np

import concourse.bacc as bacc
import concourse.mybir as mybir
from concourse import bass_utils
from concourse.tile import TileContext

# problem dims (hardcoded per spec)
B, S, D, H = 16, 2048, 512, 512
N_CORES = 8
BC = B // N_CORES  # batch rows per core
P = 128  # SBUF partitions
S_TILE = 512

F32 = mybir.dt.float32
FP16 = mybir.dt.float16
ACT = mybir.ActivationFunctionType
ALU = mybir.AluOpType


def build_nc(bc=BC, s=S, d=D, h=H, s_tile=S_TILE):
    """Build the SPMD Bass program (same program on every core)."""
    nc = bacc.Bacc("TRN2", target_bir_lowering=False)

    xt = nc.dram_tensor("xt", [bc, d, s], F32, kind="ExternalInput")
    w0f = nc.dram_tensor("w0f", [d, 3 * h], F32, kind="ExternalInput")
    w0b = nc.dram_tensor("w0b", [d, 3 * h], F32, kind="ExternalInput")
    b0f = nc.dram_tensor("b0f", [3 * h], F32, kind="ExternalInput")
    b0b = nc.dram_tensor("b0b", [3 * h], F32, kind="ExternalInput")
    w1f = nc.dram_tensor("w1f", [2 * h, 3 * h], F32, kind="ExternalInput")
    w1b = nc.dram_tensor("w1b", [2 * h, 3 * h], F32, kind="ExternalInput")
    b1f = nc.dram_tensor("b1f", [3 * h], F32, kind="ExternalInput")
    b1b = nc.dram_tensor("b1b", [3 * h], F32, kind="ExternalInput")
    out_t = nc.dram_tensor("out_t", [bc, 2 * h, s], F32, kind="ExternalOutput")

    ns = s // s_tile
    hc = h // P
    k0 = d // P       # layer-0 contraction chunks
    k1 = 2 * h // P   # layer-1 contraction chunks

    with TileContext(nc) as tc:
        with (
            tc.tile_pool(name="wpool", bufs=1) as wpool,       # weights (bf16)
            tc.tile_pool(name="wstage", bufs=1) as wstage,     # f32 weight staging
            tc.tile_pool(name="y1pool", bufs=1) as y1pool,     # inter-layer act
            tc.tile_pool(name="scr", bufs=3) as spool,         # z/f/o/g/c/y tiles
            tc.tile_pool(name="carry", bufs=1) as cpool,
            tc.tile_pool(name="instream", bufs=1) as ypool,    # layer-0 input
            tc.tile_pool(name="ps", bufs=1, space="PSUM") as ppool,
        ):
            # ---------------- weights ----------------
            # Each weight set -> k chunk tiles [P, 3h] bf16. DMAs pull TWO
            # chunks per transfer into a double stage tile; casts move one
            # chunk each so they drip into engine slack.
            wk = {
                "w0f": [wpool.tile([P, 3 * h], FP16, tag=f"w0f_{k}", name=f"w0f_{k}")
                        for k in range(k0)],
                "w0b": [wpool.tile([P, 3 * h], FP16, tag=f"w0b_{k}", name=f"w0b_{k}")
                        for k in range(k0)],
                "w1f": [wpool.tile([P, 3 * h], FP16, tag=f"w1f_{k}", name=f"w1f_{k}")
                        for k in range(k1)],
                "w1b": [wpool.tile([P, 3 * h], FP16, tag=f"w1b_{k}", name=f"w1b_{k}")
                        for k in range(k1)],
            }

            def load_biases(bd, prefix):
                """One DMA: [3h] bias -> [P, 3*hc] column table; returns
                per-(gate, h-chunk) [P, 1] views."""
                btab = wpool.tile([P, 3 * hc], F32, tag=f"{prefix}_btab",
                                  name=f"{prefix}_btab")
                nc.scalar.dma_start(btab[:], bd[:].rearrange("(j p) -> p j", p=P))
                return {
                    (g, hh): btab[:, g * hc + hh : g * hc + hh + 1]
                    for g in range(3)
                    for hh in range(hc)
                }

            # Eager w0f: per-chunk DMAs split across the two HWDGE queues,
            # casts split across scalar+vector (all idle at kernel start).
            bt = {}
            bt["w0f"] = load_biases(b0f, "b0f")  # scalar q first: gates 1st act
            w0f_stg = []
            for k in range(k0):
                stg = wstage.tile([P, 3 * h], F32, tag="w0f_stg", bufs=4,
                                  name=f"w0f_stg{k}")
                q = nc.scalar if k % 2 == 0 else nc.sync
                q.dma_start(stg[:], w0f[k * P : (k + 1) * P, :])
                w0f_stg.append(stg)
            for k in range(k0):
                eng = nc.scalar if k % 2 == 0 else nc.vector
                if k % 2 == 0:
                    eng.copy(wk["w0f"][k][:], w0f_stg[k][:])
                else:
                    eng.tensor_copy(wk["w0f"][k][:], w0f_stg[k][:])
            bt["w0b"] = load_biases(b0b, "b0b")
            bt["w1f"] = load_biases(b1f, "b1f")

            # Drip items for the other weight sets. Each DMA item loads TWO
            # chunks [P, 2*3h]; each cast item converts one chunk. Cast
            # engine rotates scalar -> gpsimd to spread the load.
            def weight_items(wd, name, k_chunks, cast_engines):
                stgs = {}

                def dma_item(kk):
                    stg = wstage.tile([P, 2 * 3 * h], F32, tag="wdrip_stg",
                                      bufs=2, name=f"{name}_stg{kk}")
                    stgs[kk] = stg
                    src = wd[kk * 2 * P : (kk + 1) * 2 * P, :].rearrange(
                        "(a p) c -> p a c", p=P)
                    dst = stg[:].rearrange("p (a c) -> p a c", a=2)
                    nc.scalar.dma_start(dst, src)

                def cast_item(k, ce):
                    stg = stgs[k // 2]
                    half = stg[:, (k % 2) * 3 * h : (k % 2 + 1) * 3 * h]
                    if ce == "scalar":
                        nc.scalar.copy(wk[name][k][:], half)
                    elif ce == "vector":
                        nc.vector.tensor_copy(wk[name][k][:], half)
                    else:
                        nc.gpsimd.tensor_copy(wk[name][k][:], half)

                items = []
                for kk in range(k_chunks // 2):
                    items.append(lambda kk=kk: dma_item(kk))
                    for j in (0, 1):
                        k = kk * 2 + j
                        ce = cast_engines[k % len(cast_engines)]
                        items.append(lambda k=k, ce=ce: cast_item(k, ce))
                return items

            drip = []
            drip += weight_items(w0b, "w0b", k0, ["scalar", "vector"])
            drip += weight_items(w1f, "w1f", k1, ["scalar", "vector"])
            drip_late = [lambda: bt.__setitem__("w1b", load_biases(b1b, "b1b"))]
            drip_late += weight_items(w1b, "w1b", k1, ["scalar", "scalar"])

            # ---------------- schedule ----------------
            # Per row: L0fw, L0bw, L1fw, L1bw. Layer-0 segs stream xt and
            # write y1 SBUF chunks; layer-1 segs read y1 chunks, write out_t.
            l0_iters = []  # flattened layer-0 iteration keys, for prefetch
            for b in range(bc):
                for fw in (True, False):
                    order = range(ns) if fw else range(ns - 1, -1, -1)
                    for si in order:
                        l0_iters.append((b, fw, si))
            l0_pos = {key: i for i, key in enumerate(l0_iters)}
            in_tiles = {}  # (b, fw, si) -> list of casted bf16 input tiles

            def issue_l0_input(key):
                """DMA + cast (gpsimd) one layer-0 input s-tile."""
                b, fw, si = key
                s0 = si * s_tile
                tiles = []
                for k in range(k0):
                    stg = ypool.tile([P, s_tile], F32, tag="instg", bufs=4,
                                     name="instg")
                    nc.sync.dma_start(
                        stg[:], xt[b, k * P : (k + 1) * P, s0 : s0 + s_tile]
                    )
                    t = ypool.tile([P, s_tile], FP16, tag=f"inr{k}", bufs=2,
                                   name=f"inr{k}")
                    nc.gpsimd.tensor_copy(t[:], stg[:])
                    tiles.append(t)
                in_tiles[key] = tiles

            y1c = {}  # chunk index 0..7 -> current row's SBUF tile

            drip_iter = iter(drip)
            drip_late_iter = iter(drip_late)

            def do_drip(it, n):
                for _ in range(n):
                    thunk = next(it, None)
                    if thunk is not None:
                        thunk()

            issue_l0_input(l0_iters[0])

            for b in range(bc):
                for layer in (0, 1):
                    for fw in (True, False):
                        wname = f"w{layer}{'f' if fw else 'b'}"
                        btile = bt[wname]
                        kch = k0 if layer == 0 else k1
                        dir_off = 0 if fw else h
                        s_order = (list(range(ns)) if fw
                                   else list(range(ns - 1, -1, -1)))
                        carry = [cpool.tile([P, 1], FP16, tag=f"c{hh}",
                                            name=f"carry{hh}")
                                 for hh in range(hc)]
                        if layer == 0 and fw:
                            # this row's y1 chunks (fw: 0..3, bw: 4..7)
                            for c in range(hc):
                                y1c[c] = y1pool.tile(
                                    [P, s], FP16, tag=f"y1c{c}", name=f"y1c{c}")
                        if layer == 0 and not fw:
                            for c in range(hc):
                                y1c[hc + c] = y1pool.tile(
                                    [P, s], FP16, tag=f"y1c{hc + c}",
                                    name=f"y1c{hc + c}")
                        for si, s_idx in enumerate(s_order):
                            s0 = s_idx * s_tile
                            if layer == 0:
                                # prefetch next layer-0 input s-tile
                                pos = l0_pos[(b, fw, s_idx)]
                                if pos + 1 < len(l0_iters):
                                    issue_l0_input(l0_iters[pos + 1])
                                ins = [t[:] for t in
                                       in_tiles.pop((b, fw, s_idx))]
                                if b == 0:
                                    do_drip(drip_iter, 3)
                            else:
                                ins = [y1c[k][:, s0 : s0 + s_tile]
                                       for k in range(k1)]
                                if b == 0:
                                    do_drip(drip_late_iter, 3)
                            for hh in range(hc):
                                ps = [
                                    ppool.tile([P, s_tile], F32, tag=f"ps{g}",
                                               name=f"ps{g}",
                                               bufs=(3 if g < 2 else 2))
                                    for g in range(3)
                                ]
                                for g in range(3):
                                    cols = slice(g * h + hh * P,
                                                 g * h + (hh + 1) * P)
                                    for k in range(kch):
                                        nc.tensor.matmul(
                                            ps[g][:],
                                            wk[wname][k][:, cols],
                                            ins[k],
                                            start=(k == 0),
                                            stop=(k == kch - 1),
                                        )
                                zn = spool.tile([P, s_tile], FP16, tag="zn",
                                                name="zn")
                                f_ = spool.tile([P, s_tile], FP16, tag="f",
                                                name="f")
                                o = spool.tile([P, s_tile], FP16, tag="o",
                                               name="o")
                                g_ = spool.tile([P, s_tile], FP16, tag="g",
                                                name="g")
                                c_ = spool.tile([P, s_tile], FP16, tag="c",
                                                name="c")
                                # host negated the z-gate W/b: Tanh yields -z
                                nc.scalar.activation(zn[:], ps[0][:], ACT.Tanh,
                                                     bias=btile[0, hh][:])
                                nc.scalar.activation(f_[:], ps[1][:],
                                                     ACT.Sigmoid,
                                                     bias=btile[1, hh][:])
                                nc.scalar.activation(o[:], ps[2][:],
                                                     ACT.Sigmoid,
                                                     bias=btile[2, hh][:])
                                # g = (f - 1) * (-z) = (1 - f) z, one fused op
                                nc.vector.scalar_tensor_tensor(
                                    g_[:], f_[:], 1.0, zn[:],
                                    ALU.subtract, ALU.mult)
                                # c_t = f_t * c_prev + g_t (bw: reversed time)
                                if fw:
                                    sc = (c_[:], f_[:], g_[:])
                                    carry_col = slice(s_tile - 1, s_tile)
                                else:
                                    sc = (c_[:, ::-1], f_[:, ::-1], g_[:, ::-1])
                                    carry_col = slice(0, 1)
                                init = 0.0 if si == 0 else carry[hh][:]
                                nc.vector.tensor_tensor_scan(
                                    sc[0], sc[1], sc[2], init,
                                    ALU.mult, ALU.add)
                                if si < ns - 1:
                                    nc.gpsimd.tensor_copy(carry[hh][:],
                                                          c_[:, carry_col])
                                if layer == 0:
                                    # y straight into the SBUF y1 chunk
                                    chunk = y1c[(0 if fw else hc) + hh]
                                    nc.gpsimd.tensor_mul(
                                        chunk[:, s0 : s0 + s_tile],
                                        o[:], c_[:])
                                else:
                                    y = spool.tile([P, s_tile], F32, tag="y",
                                                   name="y")
                                    nc.gpsimd.tensor_mul(y[:], o[:], c_[:])
                                    row0 = dir_off + hh * P
                                    nc.sync.dma_start(
                                        out_t[b, row0 : row0 + P,
                                              s0 : s0 + s_tile], y[:])
            for it in (drip_iter, drip_late_iter):
                for thunk in it:
                    thunk()

    nc.finalize()
    return nc


_NC_CACHE = {}


def _get_nc():
    if "v2" not in _NC_CACHE:
        _NC_CACHE["v2"] = build_nc()
    return _NC_CACHE["v2"]


def kernel(X, seqlens, W_fw0, b_fw0, W_bw0, b_bw0, W_fw1, b_fw1, W_bw1, b_bw1,
           mm_dtype="bf16", trace=False):
    """Full-input entry point: shards over 8 cores, returns [B, S, 2H] f32."""
    del seqlens, mm_dtype  # seqlens unused by the reference computation
    X = np.ascontiguousarray(np.asarray(X, dtype=np.float32))

    def neg_z(w):
        """Negate the z-gate block so tanh(ps+b) = -z on device."""
        w = np.array(np.asarray(w, dtype=np.float32))
        w[..., :H] *= -1.0
        return np.ascontiguousarray(w)

    weights = {
        "w0f": neg_z(W_fw0), "b0f": neg_z(b_fw0),
        "w0b": neg_z(W_bw0), "b0b": neg_z(b_bw0),
        "w1f": neg_z(W_fw1), "b1f": neg_z(b_fw1),
        "w1b": neg_z(W_bw1), "b1b": neg_z(b_bw1),
    }

    nc = _get_nc()
    in_maps = []
    for i in range(N_CORES):
        rows = X[i * BC : (i + 1) * BC]  # [BC, S, D]
        xt_i = np.ascontiguousarray(rows.transpose(0, 2, 1))  # [BC, D, S]
        in_maps.append({"xt": xt_i, **weights})

    res = bass_utils.run_bass_kernel_spmd(
        nc, in_maps, core_ids=list(range(N_CORES)), trace=trace
    )
    out = np.empty((B, S, 2 * H), dtype=np.float32)
    for i in range(N_CORES):
        out_t = res.results[i]["out_t"]  # [BC, 2H, S]
        out[i * BC : (i + 1) * BC] = out_t.transpose(0, 2, 1)
    kernel.last_results = res
    return out


# revision 6
# speedup vs baseline: 1.1373x; 1.1373x over previous
"""Bass/Trainium2 kernel for a 2-layer bidirectional QRNN (fo-pooling).

Reference computation (per layer, per direction):
    ZFO = X @ W + b            # [S, B, 3H]
    Z, F, O = split(ZFO); Z = tanh(Z); F = sigmoid(F); O = sigmoid(O)
    c_t = F_t * c_{t-1} + (1 - F_t) * Z_t        (bw direction: reversed time)
    Y_dir = O * C
    Y = concat(Y_fw, Y_bw)     # [S, B, 2H]
Two stacked layers; output is [B, S, 2H].

Sharding: data-parallel over batch. B=16 rows -> 2 rows per NeuronCore x 8.
Each core runs both layers for its 2 rows; no collectives.

v3 design (vs the fp32r/DRAM-y1 635us baseline):
- fp16 matmul operands, PRE-CAST ON THE HOST: X and all weights are fed
  to DRAM as float16, so matmul inputs DMA straight into SBUF with no
  on-chip casts and no staging at all (fp16, unlike fp32r, is a real
  DMA-able dtype). Measured matmul: 216ns/[128x128x512] (vs 233 fp32r --
  the 4-byte fp32r moving operand saturates SBUF read bandwidth).
  fp16 quantization of X/W adds ~1e-3 rel err (gate: 2e-2).
- The inter-layer activation Y1 lives entirely in SBUF as fp16 (8 chunk
  tiles [128, S] per row), never touching DRAM. Rows are processed
  b-major (L0fw, L0bw, L1fw, L1bw per row) so only one row's Y1 is live.
- Z-gate weight/bias columns are negated on the host, so the Tanh
  activation directly yields zneg = -z and the scan's g-input
  g = (1-f)*z = (f-1)*zneg is ONE fused DVE scalar_tensor_tensor.
- Post-PSUM values stay f32 (an fp16 scan measured SLOWER on DVE:
  1653ns vs 1455ns per [128,512]).
- Engine balance per layer-0 s-tile (PE: 10.4us): Scalar = 12
  activations (7.8); DVE = 4 scans + 4 fused g (9.9); GpSimd = 4 y-mults
  + carry columns (6.5). Input DMAs for s-tile i+1 issue at the START of
  s-tile i across both HWDGE queues.
- All weight DMAs are issued up front (w1b at L0bw start); they drain
  through queue slack long before first use. First matmul needs only the
  first w0f chunk + first input tile: ~3us after kernel start.

The time recurrence uses the DVE `tensor_tensor_scan` instruction
(state = f*state + g along the free axis); the bw direction runs the scan
through reversed access patterns with s-tiles processed in descending
order, chaining the carry via a [128,1] column copy.
"""

import numpy as np

import concourse.bacc as bacc
import concourse.mybir as mybir
from concourse import bass_utils
from concourse.tile import TileContext

# problem dims (hardcoded per spec)
B, S, D, H = 16, 2048, 512, 512
N_CORES = 8
BC = B // N_CORES  # batch rows per core
P = 128  # SBUF partitions
S_TILE = 512

F32 = mybir.dt.float32
FP16 = mybir.dt.float16
ACT = mybir.ActivationFunctionType
ALU = mybir.AluOpType


def build_nc(bc=BC, s=S, d=D, h=H, s_tile=S_TILE):
    """Build the SPMD Bass program (same program on every core)."""
    nc = bacc.Bacc("TRN2", target_bir_lowering=False)

    xt = nc.dram_tensor("xt", [bc, d, s], FP16, kind="ExternalInput")
    w0f = nc.dram_tensor("w0f", [d, 3 * h], FP16, kind="ExternalInput")
    w0b = nc.dram_tensor("w0b", [d, 3 * h], FP16, kind="ExternalInput")
    b0f = nc.dram_tensor("b0f", [3 * h], F32, kind="ExternalInput")
    b0b = nc.dram_tensor("b0b", [3 * h], F32, kind="ExternalInput")
    w1f = nc.dram_tensor("w1f", [2 * h, 3 * h], FP16, kind="ExternalInput")
    w1b = nc.dram_tensor("w1b", [2 * h, 3 * h], FP16, kind="ExternalInput")
    b1f = nc.dram_tensor("b1f", [3 * h], F32, kind="ExternalInput")
    b1b = nc.dram_tensor("b1b", [3 * h], F32, kind="ExternalInput")
    out_t = nc.dram_tensor("out_t", [bc, 2 * h, s], F32, kind="ExternalOutput")

    ns = s // s_tile
    hc = h // P
    k0 = d // P       # layer-0 contraction chunks
    k1 = 2 * h // P   # layer-1 contraction chunks

    with TileContext(nc) as tc:
        with (
            tc.tile_pool(name="wpool", bufs=1) as wpool,     # fp16 weights
            tc.tile_pool(name="y1pool", bufs=1) as y1pool,   # inter-layer act
            tc.tile_pool(name="scr", bufs=3) as spool,       # zn/f/o/g/c/y
            tc.tile_pool(name="carry", bufs=1) as cpool,
            tc.tile_pool(name="instream", bufs=1) as ypool,  # layer-0 input
            tc.tile_pool(name="ps", bufs=1, space="PSUM") as ppool,
        ):
            # ---------------- weights (plain fp16 DMAs) ----------------
            wk = {
                nm: [wpool.tile([P, 3 * h], FP16, tag=f"{nm}_{k}",
                                name=f"{nm}_{k}")
                     for k in range(kc)]
                for nm, kc in (("w0f", k0), ("w0b", k0),
                               ("w1f", k1), ("w1b", k1))
            }

            def load_w(name, wd, k_chunks):
                for k in range(k_chunks):
                    q = nc.scalar if k % 2 == 0 else nc.sync
                    q.dma_start(wk[name][k][:], wd[k * P : (k + 1) * P, :])

            def load_biases(bd, prefix):
                """One DMA: [3h] bias -> [P, 3*hc] column table; returns
                per-(gate, h-chunk) [P, 1] views."""
                btab = wpool.tile([P, 3 * hc], F32, tag=f"{prefix}_btab",
                                  name=f"{prefix}_btab")
                nc.scalar.dma_start(btab[:], bd[:].rearrange("(j p) -> p j", p=P))
                return {
                    (g, hh): btab[:, g * hc + hh : g * hc + hh + 1]
                    for g in range(3)
                    for hh in range(hc)
                }

            bt = {}
            bt["w0f"] = load_biases(b0f, "b0f")  # first: gates the first act
            load_w("w0f", w0f, k0)

            # ---------------- layer-0 input prefetch ----------------
            l0_iters = []
            for b in range(bc):
                for fw in (True, False):
                    order = range(ns) if fw else range(ns - 1, -1, -1)
                    for si in order:
                        l0_iters.append((b, fw, si))
            l0_pos = {key: i for i, key in enumerate(l0_iters)}
            in_tiles = {}

            def issue_l0_input(key):
                b, fw, si = key
                s0 = si * s_tile
                tiles = []
                for k in range(k0):
                    t = ypool.tile([P, s_tile], FP16, tag=f"inr{k}", bufs=2,
                                   name=f"inr{k}")
                    q = nc.sync if k % 2 == 0 else nc.scalar
                    q.dma_start(
                        t[:], xt[b, k * P : (k + 1) * P, s0 : s0 + s_tile])
                    tiles.append(t)
                in_tiles[key] = tiles

            issue_l0_input(l0_iters[0])
            # Remaining weights: queue now, they drain through slack long
            # before first use (w0b ~45us in, w1f ~90us in).
            bt["w0b"] = load_biases(b0b, "b0b")
            bt["w1f"] = load_biases(b1f, "b1f")
            load_w("w0b", w0b, k0)
            load_w("w1f", w1f, k1)

            y1c = {}  # chunk index 0..7 -> current row's SBUF tile

            for b in range(bc):
                for layer in (0, 1):
                    for fw in (True, False):
                        wname = f"w{layer}{'f' if fw else 'b'}"
                        kch = k0 if layer == 0 else k1
                        dir_off = 0 if fw else h
                        s_order = (list(range(ns)) if fw
                                   else list(range(ns - 1, -1, -1)))
                        if b == 0 and layer == 0 and not fw:
                            bt["w1b"] = load_biases(b1b, "b1b")
                            load_w("w1b", w1b, k1)
                        btile = bt[wname]
                        carry = [cpool.tile([P, 1], F32, tag=f"c{hh}",
                                            name=f"carry{hh}")
                                 for hh in range(hc)]
                        if layer == 0:
                            off = 0 if fw else hc
                            for c in range(hc):
                                y1c[off + c] = y1pool.tile(
                                    [P, s], FP16, tag=f"y1c{off + c}",
                                    name=f"y1c{off + c}")
                        for si, s_idx in enumerate(s_order):
                            s0 = s_idx * s_tile
                            if layer == 0:
                                pos = l0_pos[(b, fw, s_idx)]
                                if pos + 1 < len(l0_iters):
                                    issue_l0_input(l0_iters[pos + 1])
                                ins = [t[:] for t in
                                       in_tiles.pop((b, fw, s_idx))]
                            else:
                                ins = [y1c[k][:, s0 : s0 + s_tile]
                                       for k in range(k1)]
                            for hh in range(hc):
                                ps = [
                                    ppool.tile([P, s_tile], F32, tag=f"ps{g}",
                                               name=f"ps{g}",
                                               bufs=(3 if g < 2 else 2))
                                    for g in range(3)
                                ]
                                for g in range(3):
                                    cols = slice(g * h + hh * P,
                                                 g * h + (hh + 1) * P)
                                    for k in range(kch):
                                        nc.tensor.matmul(
                                            ps[g][:],
                                            wk[wname][k][:, cols],
                                            ins[k],
                                            start=(k == 0),
                                            stop=(k == kch - 1),
                                        )
                                zn = spool.tile([P, s_tile], F32, tag="zn",
                                                name="zn")
                                f_ = spool.tile([P, s_tile], F32, tag="f",
                                                name="f")
                                o = spool.tile([P, s_tile], F32, tag="o",
                                               name="o")
                                g_ = spool.tile([P, s_tile], F32, tag="g",
                                                name="g")
                                c_ = spool.tile([P, s_tile], F32, tag="c",
                                                name="c")
                                # host negated the z-gate W/b: Tanh gives -z
                                nc.scalar.activation(zn[:], ps[0][:], ACT.Tanh,
                                                     bias=btile[0, hh][:])
                                nc.scalar.activation(f_[:], ps[1][:],
                                                     ACT.Sigmoid,
                                                     bias=btile[1, hh][:])
                                nc.scalar.activation(o[:], ps[2][:],
                                                     ACT.Sigmoid,
                                                     bias=btile[2, hh][:])
                                # g = (f - 1) * (-z) = (1 - f) z, one DVE op
                                nc.vector.scalar_tensor_tensor(
                                    g_[:], f_[:], 1.0, zn[:],
                                    ALU.subtract, ALU.mult)
                                # c_t = f_t * c_prev + g_t (bw: reversed time)
                                if fw:
                                    sc = (c_[:], f_[:], g_[:])
                                    carry_col = slice(s_tile - 1, s_tile)
                                else:
                                    sc = (c_[:, ::-1], f_[:, ::-1], g_[:, ::-1])
                                    carry_col = slice(0, 1)
                                init = 0.0 if si == 0 else carry[hh][:]
                                nc.vector.tensor_tensor_scan(
                                    sc[0], sc[1], sc[2], init,
                                    ALU.mult, ALU.add)
                                if si < ns - 1:
                                    nc.gpsimd.tensor_copy(carry[hh][:],
                                                          c_[:, carry_col])
                                if layer == 0:
                                    chunk = y1c[(0 if fw else hc) + hh]
                                    nc.gpsimd.tensor_mul(
                                        chunk[:, s0 : s0 + s_tile],
                                        o[:], c_[:])
                                else:
                                    y = spool.tile([P, s_tile], F32, tag="y",
                                                   name="y")
                                    nc.gpsimd.tensor_mul(y[:], o[:], c_[:])
                                    row0 = dir_off + hh * P
                                    nc.sync.dma_start(
                                        out_t[b, row0 : row0 + P,
                                              s0 : s0 + s_tile], y[:])

    nc.finalize()
    return nc


_NC_CACHE = {}


def _get_nc():
    if "v3" not in _NC_CACHE:
        _NC_CACHE["v3"] = build_nc()
    return _NC_CACHE["v3"]


def kernel(X, seqlens, W_fw0, b_fw0, W_bw0, b_bw0, W_fw1, b_fw1, W_bw1, b_bw1,
           mm_dtype="fp16", trace=False):
    """Full-input entry point: shards over 8 cores, returns [B, S, 2H] f32."""
    del seqlens, mm_dtype  # seqlens unused by the reference computation
    X = np.asarray(X, dtype=np.float32)

    def neg_z(v, dt):
        """Negate the z-gate block so tanh(ps+b) = -z on device."""
        v = np.array(np.asarray(v, dtype=np.float32))
        v[..., :H] *= -1.0
        return np.ascontiguousarray(v.astype(dt))

    weights = {
        "w0f": neg_z(W_fw0, np.float16), "b0f": neg_z(b_fw0, np.float32),
        "w0b": neg_z(W_bw0, np.float16), "b0b": neg_z(b_bw0, np.float32),
        "w1f": neg_z(W_fw1, np.float16), "b1f": neg_z(b_fw1, np.float32),
        "w1b": neg_z(W_bw1, np.float16), "b1b": neg_z(b_bw1, np.float32),
    }

    nc = _get_nc()
    in_maps = []
    for i in range(N_CORES):
        rows = X[i * BC : (i + 1) * BC]  # [BC, S, D]
        xt_i = np.ascontiguousarray(
            rows.transpose(0, 2, 1).astype(np.float16))  # [BC, D, S] fp16
        in_maps.append({"xt": xt_i, **weights})

    res = bass_utils.run_bass_kernel_spmd(
        nc, in_maps, core_ids=list(range(N_CORES)), trace=trace
    )
    out = np.empty((B, S, 2 * H), dtype=np.float32)
    for i in range(N_CORES):
        out_t = res.results[i]["out_t"]  # [BC, 2H, S]
        out[i * BC : (i + 1) * BC] = out_t.transpose(0, 2, 1)
    kernel.last_results = res
    return out


# revision 7
# speedup vs baseline: 1.1973x; 1.0528x over previous
"""Bass/Trainium2 kernel for a 2-layer bidirectional QRNN (fo-pooling).

Reference computation (per layer, per direction):
    ZFO = X @ W + b            # [S, B, 3H]
    Z, F, O = split(ZFO); Z = tanh(Z); F = sigmoid(F); O = sigmoid(O)
    c_t = F_t * c_{t-1} + (1 - F_t) * Z_t        (bw direction: reversed time)
    Y_dir = O * C
    Y = concat(Y_fw, Y_bw)     # [S, B, 2H]
Two stacked layers; output is [B, S, 2H].

Sharding: data-parallel over batch. B=16 rows -> 2 rows per NeuronCore x 8.
Each core runs both layers for its 2 rows; no collectives.

v3 design (vs the fp32r/DRAM-y1 635us baseline):
- fp16 matmul operands, PRE-CAST ON THE HOST: X and all weights are fed
  to DRAM as float16, so matmul inputs DMA straight into SBUF with no
  on-chip casts and no staging at all (fp16, unlike fp32r, is a real
  DMA-able dtype). Measured matmul: 216ns/[128x128x512] (vs 233 fp32r --
  the 4-byte fp32r moving operand saturates SBUF read bandwidth).
  fp16 quantization of X/W adds ~1e-3 rel err (gate: 2e-2).
- The inter-layer activation Y1 lives entirely in SBUF as fp16 (8 chunk
  tiles [128, S] per row), never touching DRAM. Rows are processed
  b-major (L0fw, L0bw, L1fw, L1bw per row) so only one row's Y1 is live.
- Z-gate weight/bias columns are negated on the host, so the Tanh
  activation directly yields zneg = -z and the scan's g-input
  g = (1-f)*z = (f-1)*zneg is ONE fused DVE scalar_tensor_tensor.
- Post-PSUM values stay f32 (an fp16 scan measured SLOWER on DVE:
  1653ns vs 1455ns per [128,512]).
- Engine balance per layer-0 s-tile (PE: 10.4us): Scalar = 12
  activations (7.8); DVE = 4 scans + 4 fused g (9.9); GpSimd = 4 y-mults
  + carry columns (6.5). Input DMAs for s-tile i+1 issue at the START of
  s-tile i across both HWDGE queues.
- All weight DMAs are issued up front (w1b at L0bw start); they drain
  through queue slack long before first use. First matmul needs only the
  first w0f chunk + first input tile: ~3us after kernel start.

The time recurrence uses the DVE `tensor_tensor_scan` instruction
(state = f*state + g along the free axis); the bw direction runs the scan
through reversed access patterns with s-tiles processed in descending
order, chaining the carry via a [128,1] column copy.
"""

import numpy as np

import concourse.bacc as bacc
import concourse.mybir as mybir
from concourse import bass_utils
from concourse.tile import TileContext

# problem dims (hardcoded per spec)
B, S, D, H = 16, 2048, 512, 512
N_CORES = 8
BC = B // N_CORES  # batch rows per core
P = 128  # SBUF partitions
S_TILE = 512

F32 = mybir.dt.float32
FP16 = mybir.dt.float16
ACT = mybir.ActivationFunctionType
ALU = mybir.AluOpType


def build_nc(bc=BC, s=S, d=D, h=H, s_tile=S_TILE):
    """Build the SPMD Bass program (same program on every core)."""
    nc = bacc.Bacc("TRN2", target_bir_lowering=False)

    xt = nc.dram_tensor("xt", [bc, d, s], FP16, kind="ExternalInput")
    w0f = nc.dram_tensor("w0f", [d, 3 * h], FP16, kind="ExternalInput")
    w0b = nc.dram_tensor("w0b", [d, 3 * h], FP16, kind="ExternalInput")
    b0f = nc.dram_tensor("b0f", [P, 3 * (h // P)], F32, kind="ExternalInput")
    b0b = nc.dram_tensor("b0b", [P, 3 * (h // P)], F32, kind="ExternalInput")
    w1f = nc.dram_tensor("w1f", [2 * h, 3 * h], FP16, kind="ExternalInput")
    w1b = nc.dram_tensor("w1b", [2 * h, 3 * h], FP16, kind="ExternalInput")
    b1f = nc.dram_tensor("b1f", [P, 3 * (h // P)], F32, kind="ExternalInput")
    b1b = nc.dram_tensor("b1b", [P, 3 * (h // P)], F32, kind="ExternalInput")
    out_t = nc.dram_tensor("out_t", [bc, 2 * h, s], F32, kind="ExternalOutput")

    ns = s // s_tile
    hc = h // P
    k0 = d // P       # layer-0 contraction chunks
    k1 = 2 * h // P   # layer-1 contraction chunks

    with TileContext(nc) as tc:
        with (
            tc.tile_pool(name="wpool", bufs=1) as wpool,     # fp16 weights
            tc.tile_pool(name="y1pool", bufs=1) as y1pool,   # inter-layer act
            tc.tile_pool(name="scr", bufs=3) as spool,       # zn/f/o/g/c/y
            tc.tile_pool(name="carry", bufs=1) as cpool,
            tc.tile_pool(name="instream", bufs=1) as ypool,  # layer-0 input
            tc.tile_pool(name="ps", bufs=1, space="PSUM") as ppool,
        ):
            # ---------------- weights (plain fp16 DMAs) ----------------
            wk = {
                nm: [wpool.tile([P, 3 * h], FP16, tag=f"{nm}_{k}",
                                name=f"{nm}_{k}")
                     for k in range(kc)]
                for nm, kc in (("w0f", k0), ("w0b", k0),
                               ("w1f", k1), ("w1b", k1))
            }

            def load_w(name, wd, k_chunks):
                for k in range(k_chunks):
                    q = nc.scalar if k % 2 == 0 else nc.sync
                    q.dma_start(wk[name][k][:], wd[k * P : (k + 1) * P, :])

            def load_biases(bd, prefix):
                """One plain DMA of the host-pretabled [P, 3*hc] bias;
                returns per-(gate, h-chunk) [P, 1] views."""
                btab = wpool.tile([P, 3 * hc], F32, tag=f"{prefix}_btab",
                                  name=f"{prefix}_btab")
                nc.scalar.dma_start(btab[:], bd[:, :])
                return {
                    (g, hh): btab[:, g * hc + hh : g * hc + hh + 1]
                    for g in range(3)
                    for hh in range(hc)
                }

            bt = {}
            bt["w0f"] = load_biases(b0f, "b0f")  # first: gates the first act
            load_w("w0f", w0f, k0)

            # ---------------- layer-0 input prefetch ----------------
            l0_iters = []
            for b in range(bc):
                for fw in (True, False):
                    order = range(ns) if fw else range(ns - 1, -1, -1)
                    for si in order:
                        l0_iters.append((b, fw, si))
            l0_pos = {key: i for i, key in enumerate(l0_iters)}
            in_tiles = {}

            def issue_l0_input(key):
                b, fw, si = key
                s0 = si * s_tile
                tiles = []
                for k in range(k0):
                    t = ypool.tile([P, s_tile], FP16, tag=f"inr{k}", bufs=2,
                                   name=f"inr{k}")
                    q = nc.sync if k % 2 == 0 else nc.scalar
                    q.dma_start(
                        t[:], xt[b, k * P : (k + 1) * P, s0 : s0 + s_tile])
                    tiles.append(t)
                in_tiles[key] = tiles

            issue_l0_input(l0_iters[0])
            # Remaining weights: queue now, they drain through slack long
            # before first use (w0b ~45us in, w1f ~90us in).
            bt["w0b"] = load_biases(b0b, "b0b")
            bt["w1f"] = load_biases(b1f, "b1f")
            load_w("w0b", w0b, k0)
            load_w("w1f", w1f, k1)

            y1c = {}  # chunk index 0..7 -> current row's SBUF tile

            for b in range(bc):
                for layer in (0, 1):
                    for fw in ((True, False) if layer == 0 else (False, True)):
                        wname = f"w{layer}{'f' if fw else 'b'}"
                        kch = k0 if layer == 0 else k1
                        dir_off = 0 if fw else h
                        s_order = (list(range(ns)) if fw
                                   else list(range(ns - 1, -1, -1)))
                        if b == 0 and layer == 0 and not fw:
                            bt["w1b"] = load_biases(b1b, "b1b")
                            load_w("w1b", w1b, k1)
                        btile = bt[wname]
                        carry = [cpool.tile([P, 1], F32, tag=f"c{hh}",
                                            name=f"carry{hh}")
                                 for hh in range(hc)]
                        if layer == 0:
                            off = 0 if fw else hc
                            for c in range(hc):
                                y1c[off + c] = y1pool.tile(
                                    [P, s], FP16, tag=f"y1c{off + c}",
                                    name=f"y1c{off + c}")
                        for si, s_idx in enumerate(s_order):
                            s0 = s_idx * s_tile
                            if layer == 0:
                                pos = l0_pos[(b, fw, s_idx)]
                                if pos + 1 < len(l0_iters):
                                    issue_l0_input(l0_iters[pos + 1])
                                ins = [t[:] for t in
                                       in_tiles.pop((b, fw, s_idx))]
                            else:
                                ins = [y1c[k][:, s0 : s0 + s_tile]
                                       for k in range(k1)]
                            for hh in range(hc):
                                ps = [
                                    ppool.tile([P, s_tile], F32, tag=f"ps{g}",
                                               name=f"ps{g}",
                                               bufs=(3 if g < 2 else 2))
                                    for g in range(3)
                                ]
                                for g in range(3):
                                    cols = slice(g * h + hh * P,
                                                 g * h + (hh + 1) * P)
                                    for k in range(kch):
                                        nc.tensor.matmul(
                                            ps[g][:],
                                            wk[wname][k][:, cols],
                                            ins[k],
                                            start=(k == 0),
                                            stop=(k == kch - 1),
                                        )
                                zn = spool.tile([P, s_tile], F32, tag="zn",
                                                name="zn")
                                f_ = spool.tile([P, s_tile], F32, tag="f",
                                                name="f")
                                o = spool.tile([P, s_tile], F32, tag="o",
                                               name="o")
                                g_ = spool.tile([P, s_tile], F32, tag="g",
                                                name="g")
                                c_ = spool.tile([P, s_tile], F32, tag="c",
                                                name="c")
                                # host negated the z-gate W/b: Tanh gives -z
                                nc.scalar.activation(zn[:], ps[0][:], ACT.Tanh,
                                                     bias=btile[0, hh][:])
                                nc.scalar.activation(f_[:], ps[1][:],
                                                     ACT.Sigmoid,
                                                     bias=btile[1, hh][:])
                                nc.scalar.activation(o[:], ps[2][:],
                                                     ACT.Sigmoid,
                                                     bias=btile[2, hh][:])
                                # g = (f - 1) * (-z) = (1 - f) z, one DVE op
                                nc.vector.scalar_tensor_tensor(
                                    g_[:], f_[:], 1.0, zn[:],
                                    ALU.subtract, ALU.mult)
                                # c_t = f_t * c_prev + g_t (bw: reversed time)
                                if fw:
                                    sc = (c_[:], f_[:], g_[:])
                                    carry_col = slice(s_tile - 1, s_tile)
                                else:
                                    sc = (c_[:, ::-1], f_[:, ::-1], g_[:, ::-1])
                                    carry_col = slice(0, 1)
                                init = 0.0 if si == 0 else carry[hh][:]
                                nc.vector.tensor_tensor_scan(
                                    sc[0], sc[1], sc[2], init,
                                    ALU.mult, ALU.add)
                                if si < ns - 1:
                                    nc.gpsimd.tensor_copy(carry[hh][:],
                                                          c_[:, carry_col])
                                if layer == 0:
                                    chunk = y1c[(0 if fw else hc) + hh]
                                    nc.gpsimd.tensor_mul(
                                        chunk[:, s0 : s0 + s_tile],
                                        o[:], c_[:])
                                else:
                                    y = spool.tile([P, s_tile], F32, tag="y",
                                                   name="y")
                                    nc.gpsimd.tensor_mul(y[:], o[:], c_[:])
                                    row0 = dir_off + hh * P
                                    nc.sync.dma_start(
                                        out_t[b, row0 : row0 + P,
                                              s0 : s0 + s_tile], y[:])

    nc.finalize()
    return nc


_NC_CACHE = {}


def _get_nc():
    if "v3" not in _NC_CACHE:
        _NC_CACHE["v3"] = build_nc()
    return _NC_CACHE["v3"]


def kernel(X, seqlens, W_fw0, b_fw0, W_bw0, b_bw0, W_fw1, b_fw1, W_bw1, b_bw1,
           mm_dtype="fp16", trace=False):
    """Full-input entry point: shards over 8 cores, returns [B, S, 2H] f32."""
    del seqlens, mm_dtype  # seqlens unused by the reference computation
    X = np.asarray(X, dtype=np.float32)

    def neg_z(v, dt):
        """Negate the z-gate block so tanh(ps+b) = -z on device."""
        v = np.array(np.asarray(v, dtype=np.float32))
        v[..., :H] *= -1.0
        return np.ascontiguousarray(v.astype(dt))

    def btab(v):
        """[3H] bias -> [P, 3*H/P] column table (z-gate negated)."""
        v = neg_z(v, np.float32)  # [3H]
        t = v.reshape(3 * H // P, P).T  # [P, 3hc]
        return np.ascontiguousarray(t)

    weights = {
        "w0f": neg_z(W_fw0, np.float16), "b0f": btab(b_fw0),
        "w0b": neg_z(W_bw0, np.float16), "b0b": btab(b_bw0),
        "w1f": neg_z(W_fw1, np.float16), "b1f": btab(b_fw1),
        "w1b": neg_z(W_bw1, np.float16), "b1b": btab(b_bw1),
    }

    nc = _get_nc()
    in_maps = []
    for i in range(N_CORES):
        rows = X[i * BC : (i + 1) * BC]  # [BC, S, D]
        xt_i = np.ascontiguousarray(
            rows.transpose(0, 2, 1).astype(np.float16))  # [BC, D, S] fp16
        in_maps.append({"xt": xt_i, **weights})

    res = bass_utils.run_bass_kernel_spmd(
        nc, in_maps, core_ids=list(range(N_CORES)), trace=trace
    )
    out = np.empty((B, S, 2 * H), dtype=np.float32)
    for i in range(N_CORES):
        out_t = res.results[i]["out_t"]  # [BC, 2H, S]
        out[i * BC : (i + 1) * BC] = out_t.transpose(0, 2, 1)
    kernel.last_results = res
    return out


# revision 8
# speedup vs baseline: 1.1977x; 1.0003x over previous
"""Bass/Trainium2 kernel for a 2-layer bidirectional QRNN (fo-pooling).

Reference computation (per layer, per direction):
    ZFO = X @ W + b            # [S, B, 3H]
    Z, F, O = split(ZFO); Z = tanh(Z); F = sigmoid(F); O = sigmoid(O)
    c_t = F_t * c_{t-1} + (1 - F_t) * Z_t        (bw direction: reversed time)
    Y_dir = O * C
    Y = concat(Y_fw, Y_bw)     # [S, B, 2H]
Two stacked layers; output is [B, S, 2H].

Sharding: data-parallel over batch. B=16 rows -> 2 rows per NeuronCore x 8.
Each core runs both layers for its 2 rows; no collectives.

v3 design (vs the fp32r/DRAM-y1 635us baseline):
- fp16 matmul operands, PRE-CAST ON THE HOST: X and all weights are fed
  to DRAM as float16, so matmul inputs DMA straight into SBUF with no
  on-chip casts and no staging at all (fp16, unlike fp32r, is a real
  DMA-able dtype). Measured matmul: 216ns/[128x128x512] (vs 233 fp32r --
  the 4-byte fp32r moving operand saturates SBUF read bandwidth).
  fp16 quantization of X/W adds ~1e-3 rel err (gate: 2e-2).
- The inter-layer activation Y1 lives entirely in SBUF as fp16 (8 chunk
  tiles [128, S] per row), never touching DRAM. Rows are processed
  b-major (L0fw, L0bw, L1fw, L1bw per row) so only one row's Y1 is live.
- Z-gate weight/bias columns are negated on the host, so the Tanh
  activation directly yields zneg = -z and the scan's g-input
  g = (1-f)*z = (f-1)*zneg is ONE fused DVE scalar_tensor_tensor.
- Post-PSUM values stay f32 (an fp16 scan measured SLOWER on DVE:
  1653ns vs 1455ns per [128,512]).
- Engine balance per layer-0 s-tile (PE: 10.4us): Scalar = 12
  activations (7.8); DVE = 4 scans + 4 fused g (9.9); GpSimd = 4 y-mults
  + carry columns (6.5). Input DMAs for s-tile i+1 issue at the START of
  s-tile i across both HWDGE queues.
- All weight DMAs are issued up front (w1b at L0bw start); they drain
  through queue slack long before first use. First matmul needs only the
  first w0f chunk + first input tile: ~3us after kernel start.

The time recurrence uses the DVE `tensor_tensor_scan` instruction
(state = f*state + g along the free axis); the bw direction runs the scan
through reversed access patterns with s-tiles processed in descending
order, chaining the carry via a [128,1] column copy.
"""

import numpy as np

import concourse.bacc as bacc
import concourse.mybir as mybir
from concourse import bass_utils
from concourse.tile import TileContext

# problem dims (hardcoded per spec)
B, S, D, H = 16, 2048, 512, 512
N_CORES = 8
BC = B // N_CORES  # batch rows per core
P = 128  # SBUF partitions
S_TILE = 512

F32 = mybir.dt.float32
FP16 = mybir.dt.float16
ACT = mybir.ActivationFunctionType
ALU = mybir.AluOpType


def build_nc(bc=BC, s=S, d=D, h=H, s_tile=S_TILE):
    """Build the SPMD Bass program (same program on every core)."""
    nc = bacc.Bacc("TRN2", target_bir_lowering=False)

    xt = nc.dram_tensor("xt", [bc, d, s], FP16, kind="ExternalInput")
    w0f = nc.dram_tensor("w0f", [d, 3 * h], FP16, kind="ExternalInput")
    w0b = nc.dram_tensor("w0b", [d, 3 * h], FP16, kind="ExternalInput")
    b0f = nc.dram_tensor("b0f", [P, 3 * (h // P)], F32, kind="ExternalInput")
    b0b = nc.dram_tensor("b0b", [P, 3 * (h // P)], F32, kind="ExternalInput")
    w1f = nc.dram_tensor("w1f", [2 * h, 3 * h], FP16, kind="ExternalInput")
    w1b = nc.dram_tensor("w1b", [2 * h, 3 * h], FP16, kind="ExternalInput")
    b1f = nc.dram_tensor("b1f", [P, 3 * (h // P)], F32, kind="ExternalInput")
    b1b = nc.dram_tensor("b1b", [P, 3 * (h // P)], F32, kind="ExternalInput")
    out_t = nc.dram_tensor("out_t", [bc, 2 * h, s], F32, kind="ExternalOutput")

    ns = s // s_tile
    hc = h // P
    k0 = d // P       # layer-0 contraction chunks
    k1 = 2 * h // P   # layer-1 contraction chunks

    with TileContext(nc) as tc:
        with (
            tc.tile_pool(name="wpool", bufs=1) as wpool,     # fp16 weights
            tc.tile_pool(name="y1pool", bufs=1) as y1pool,   # inter-layer act
            tc.tile_pool(name="scr", bufs=3) as spool,       # zn/f/o/g/c/y
            tc.tile_pool(name="carry", bufs=1) as cpool,
            tc.tile_pool(name="instream", bufs=1) as ypool,  # layer-0 input
            tc.tile_pool(name="ps", bufs=1, space="PSUM") as ppool,
        ):
            # ---------------- weights (plain fp16 DMAs) ----------------
            wk = {
                nm: [wpool.tile([P, 3 * h], FP16, tag=f"{nm}_{k}",
                                name=f"{nm}_{k}")
                     for k in range(kc)]
                for nm, kc in (("w0f", k0), ("w0b", k0),
                               ("w1f", k1), ("w1b", k1))
            }

            def load_w(name, wd, k_chunks):
                for k in range(k_chunks):
                    q = nc.scalar if k % 2 == 0 else nc.sync
                    q.dma_start(wk[name][k][:], wd[k * P : (k + 1) * P, :])

            def load_biases(bd, prefix):
                """One plain DMA of the host-pretabled [P, 3*hc] bias;
                returns per-(gate, h-chunk) [P, 1] views."""
                btab = wpool.tile([P, 3 * hc], F32, tag=f"{prefix}_btab",
                                  name=f"{prefix}_btab")
                nc.scalar.dma_start(btab[:], bd[:, :])
                return {
                    (g, hh): btab[:, g * hc + hh : g * hc + hh + 1]
                    for g in range(3)
                    for hh in range(hc)
                }

            # PE warm-up: dummy matmuls on memset scratch keep the PE
            # busy through the DMA bring-up window, so the p-state governor
            # is at full clock when the first real matmul issues.
            warm_w = wpool.tile([P, P], FP16, tag="warm_w", name="warm_w")
            warm_m = wpool.tile([P, s_tile], FP16, tag="warm_m", name="warm_m")
            nc.gpsimd.memset(warm_w[:], 0.0)
            nc.gpsimd.memset(warm_m[:], 0.0)
            warm_ps = ppool.tile([P, s_tile], F32, tag="ps2", name="warm_ps",
                                 bufs=2)
            for _ in range(30):
                nc.tensor.matmul(warm_ps[:], warm_w[:], warm_m[:],
                                 start=True, stop=True)

            bt = {}
            # Queue order at startup: the first matmul needs w0f chunk 0
            # (scalar q) + input tile 0 (sync q); everything else comes
            # after those on their queues.
            load_w("w0f", w0f, k0)

            # ---------------- layer-0 input prefetch ----------------
            l0_iters = []
            for b in range(bc):
                for fw in (True, False):
                    order = range(ns) if fw else range(ns - 1, -1, -1)
                    for si in order:
                        l0_iters.append((b, fw, si))
            l0_pos = {key: i for i, key in enumerate(l0_iters)}
            in_tiles = {}

            def issue_l0_input(key):
                b, fw, si = key
                s0 = si * s_tile
                tiles = []
                for k in range(k0):
                    t = ypool.tile([P, s_tile], FP16, tag=f"inr{k}", bufs=2,
                                   name=f"inr{k}")
                    q = nc.sync if k % 2 == 0 else nc.scalar
                    q.dma_start(
                        t[:], xt[b, k * P : (k + 1) * P, s0 : s0 + s_tile])
                    tiles.append(t)
                in_tiles[key] = tiles

            issue_l0_input(l0_iters[0])
            # Remaining weights: queue now, they drain through slack long
            # before first use (w0b ~45us in, w1f ~90us in).
            bt["w0f"] = load_biases(b0f, "b0f")
            bt["w0b"] = load_biases(b0b, "b0b")
            bt["w1f"] = load_biases(b1f, "b1f")
            load_w("w0b", w0b, k0)
            load_w("w1f", w1f, k1)

            y1c = {}  # chunk index 0..7 -> current row's SBUF tile

            for b in range(bc):
                for layer in (0, 1):
                    for fw in ((True, False) if layer == 0 else (False, True)):
                        wname = f"w{layer}{'f' if fw else 'b'}"
                        kch = k0 if layer == 0 else k1
                        dir_off = 0 if fw else h
                        s_order = (list(range(ns)) if fw
                                   else list(range(ns - 1, -1, -1)))
                        if b == 0 and layer == 0 and not fw:
                            bt["w1b"] = load_biases(b1b, "b1b")
                            load_w("w1b", w1b, k1)
                        btile = bt[wname]
                        carry = [cpool.tile([P, 1], F32, tag=f"c{hh}",
                                            name=f"carry{hh}")
                                 for hh in range(hc)]
                        if layer == 0:
                            off = 0 if fw else hc
                            for c in range(hc):
                                y1c[off + c] = y1pool.tile(
                                    [P, s], FP16, tag=f"y1c{off + c}",
                                    name=f"y1c{off + c}")
                        for si, s_idx in enumerate(s_order):
                            s0 = s_idx * s_tile
                            if layer == 0:
                                pos = l0_pos[(b, fw, s_idx)]
                                if pos + 1 < len(l0_iters):
                                    issue_l0_input(l0_iters[pos + 1])
                                ins = [t[:] for t in
                                       in_tiles.pop((b, fw, s_idx))]
                            else:
                                ins = [y1c[k][:, s0 : s0 + s_tile]
                                       for k in range(k1)]
                            for hh in range(hc):
                                ps = [
                                    ppool.tile([P, s_tile], F32, tag=f"ps{g}",
                                               name=f"ps{g}",
                                               bufs=(3 if g < 2 else 2))
                                    for g in range(3)
                                ]
                                for g in range(3):
                                    cols = slice(g * h + hh * P,
                                                 g * h + (hh + 1) * P)
                                    for k in range(kch):
                                        nc.tensor.matmul(
                                            ps[g][:],
                                            wk[wname][k][:, cols],
                                            ins[k],
                                            start=(k == 0),
                                            stop=(k == kch - 1),
                                        )
                                zn = spool.tile([P, s_tile], F32, tag="zn",
                                                name="zn")
                                f_ = spool.tile([P, s_tile], F32, tag="f",
                                                name="f")
                                o = spool.tile([P, s_tile], F32, tag="o",
                                               name="o")
                                g_ = spool.tile([P, s_tile], F32, tag="g",
                                                name="g")
                                c_ = spool.tile([P, s_tile], F32, tag="c",
                                                name="c")
                                # host negated the z-gate W/b: Tanh gives -z
                                nc.scalar.activation(zn[:], ps[0][:], ACT.Tanh,
                                                     bias=btile[0, hh][:])
                                nc.scalar.activation(f_[:], ps[1][:],
                                                     ACT.Sigmoid,
                                                     bias=btile[1, hh][:])
                                nc.scalar.activation(o[:], ps[2][:],
                                                     ACT.Sigmoid,
                                                     bias=btile[2, hh][:])
                                # g = (f - 1) * (-z) = (1 - f) z, one DVE op
                                nc.vector.scalar_tensor_tensor(
                                    g_[:], f_[:], 1.0, zn[:],
                                    ALU.subtract, ALU.mult)
                                # c_t = f_t * c_prev + g_t (bw: reversed time)
                                if fw:
                                    sc = (c_[:], f_[:], g_[:])
                                    carry_col = slice(s_tile - 1, s_tile)
                                else:
                                    sc = (c_[:, ::-1], f_[:, ::-1], g_[:, ::-1])
                                    carry_col = slice(0, 1)
                                init = 0.0 if si == 0 else carry[hh][:]
                                nc.vector.tensor_tensor_scan(
                                    sc[0], sc[1], sc[2], init,
                                    ALU.mult, ALU.add)
                                if si < ns - 1:
                                    nc.gpsimd.tensor_copy(carry[hh][:],
                                                          c_[:, carry_col])
                                if layer == 0:
                                    chunk = y1c[(0 if fw else hc) + hh]
                                    nc.gpsimd.tensor_mul(
                                        chunk[:, s0 : s0 + s_tile],
                                        o[:], c_[:])
                                else:
                                    y = spool.tile([P, s_tile], F32, tag="y",
                                                   name="y")
                                    nc.gpsimd.tensor_mul(y[:], o[:], c_[:])
                                    row0 = dir_off + hh * P
                                    nc.sync.dma_start(
                                        out_t[b, row0 : row0 + P,
                                              s0 : s0 + s_tile], y[:])

    nc.finalize()
    return nc


_NC_CACHE = {}


def _get_nc():
    if "v3" not in _NC_CACHE:
        _NC_CACHE["v3"] = build_nc()
    return _NC_CACHE["v3"]


def kernel(X, seqlens, W_fw0, b_fw0, W_bw0, b_bw0, W_fw1, b_fw1, W_bw1, b_bw1,
           mm_dtype="fp16", trace=False):
    """Full-input entry point: shards over 8 cores, returns [B, S, 2H] f32."""
    del seqlens, mm_dtype  # seqlens unused by the reference computation
    X = np.asarray(X, dtype=np.float32)

    def neg_z(v, dt):
        """Negate the z-gate block so tanh(ps+b) = -z on device."""
        v = np.array(np.asarray(v, dtype=np.float32))
        v[..., :H] *= -1.0
        return np.ascontiguousarray(v.astype(dt))

    def btab(v):
        """[3H] bias -> [P, 3*H/P] column table (z-gate negated)."""
        v = neg_z(v, np.float32)  # [3H]
        t = v.reshape(3 * H // P, P).T  # [P, 3hc]
        return np.ascontiguousarray(t)

    weights = {
        "w0f": neg_z(W_fw0, np.float16), "b0f": btab(b_fw0),
        "w0b": neg_z(W_bw0, np.float16), "b0b": btab(b_bw0),
        "w1f": neg_z(W_fw1, np.float16), "b1f": btab(b_fw1),
        "w1b": neg_z(W_bw1, np.float16), "b1b": btab(b_bw1),
    }

    nc = _get_nc()
    in_maps = []
    for i in range(N_CORES):
        rows = X[i * BC : (i + 1) * BC]  # [BC, S, D]
        xt_i = np.ascontiguousarray(
            rows.transpose(0, 2, 1).astype(np.float16))  # [BC, D, S] fp16
        in_maps.append({"xt": xt_i, **weights})

    res = bass_utils.run_bass_kernel_spmd(
        nc, in_maps, core_ids=list(range(N_CORES)), trace=trace
    )
    out = np.empty((B, S, 2 * H), dtype=np.float32)
    for i in range(N_CORES):
        out_t = res.results[i]["out_t"]  # [BC, 2H, S]
        out[i * BC : (i + 1) * BC] = out_t.transpose(0, 2, 1)
    kernel.last_results = res
    return out


# revision 10
# speedup vs baseline: 1.2146x; 1.0141x over previous
"""Bass/Trainium2 kernel for a 2-layer bidirectional QRNN (fo-pooling).

Reference computation (per layer, per direction):
    ZFO = X @ W + b            # [S, B, 3H]
    Z, F, O = split(ZFO); Z = tanh(Z); F = sigmoid(F); O = sigmoid(O)
    c_t = F_t * c_{t-1} + (1 - F_t) * Z_t        (bw direction: reversed time)
    Y_dir = O * C
    Y = concat(Y_fw, Y_bw)     # [S, B, 2H]
Two stacked layers; output is [B, S, 2H].

Sharding: data-parallel over batch. B=16 rows -> 2 rows per NeuronCore x 8.
Each core runs both layers for its 2 rows; no collectives.

v3 design (vs the fp32r/DRAM-y1 635us baseline):
- fp16 matmul operands, PRE-CAST ON THE HOST: X and all weights are fed
  to DRAM as float16, so matmul inputs DMA straight into SBUF with no
  on-chip casts and no staging at all (fp16, unlike fp32r, is a real
  DMA-able dtype). Measured matmul: 216ns/[128x128x512] (vs 233 fp32r --
  the 4-byte fp32r moving operand saturates SBUF read bandwidth).
  fp16 quantization of X/W adds ~1e-3 rel err (gate: 2e-2).
- The inter-layer activation Y1 lives entirely in SBUF as fp16 (8 chunk
  tiles [128, S] per row), never touching DRAM. Rows are processed
  b-major (L0fw, L0bw, L1fw, L1bw per row) so only one row's Y1 is live.
- Z-gate weight/bias columns are negated on the host, so the Tanh
  activation directly yields zneg = -z and the scan's g-input
  g = (1-f)*z = (f-1)*zneg is ONE fused DVE scalar_tensor_tensor.
- Post-PSUM values stay f32 (an fp16 scan measured SLOWER on DVE:
  1653ns vs 1455ns per [128,512]).
- Engine balance per layer-0 s-tile (PE: 10.4us): Scalar = 12
  activations (7.8); DVE = 4 scans + 4 fused g (9.9); GpSimd = 4 y-mults
  + carry columns (6.5). Input DMAs for s-tile i+1 issue at the START of
  s-tile i across both HWDGE queues.
- All weight DMAs are issued up front (w1b at L0bw start); they drain
  through queue slack long before first use. First matmul needs only the
  first w0f chunk + first input tile: ~3us after kernel start.

The time recurrence uses the DVE `tensor_tensor_scan` instruction
(state = f*state + g along the free axis); the bw direction runs the scan
through reversed access patterns with s-tiles processed in descending
order, chaining the carry via a [128,1] column copy.
"""

import numpy as np

import concourse.bacc as bacc
import concourse.mybir as mybir
from concourse import bass_utils
from concourse.tile import TileContext

# problem dims (hardcoded per spec)
B, S, D, H = 16, 2048, 512, 512
N_CORES = 8
BC = B // N_CORES  # batch rows per core
P = 128  # SBUF partitions
S_TILE = 512

F32 = mybir.dt.float32
FP16 = mybir.dt.float16
ACT = mybir.ActivationFunctionType
ALU = mybir.AluOpType


def build_nc(bc=BC, s=S, d=D, h=H, s_tile=S_TILE):
    """Build the SPMD Bass program (same program on every core)."""
    nc = bacc.Bacc("TRN2", target_bir_lowering=False)

    xt = nc.dram_tensor("xt", [bc, d, s], FP16, kind="ExternalInput")
    w0f = nc.dram_tensor("w0f", [d, 3 * h], FP16, kind="ExternalInput")
    w0b = nc.dram_tensor("w0b", [d, 3 * h], FP16, kind="ExternalInput")
    b0f = nc.dram_tensor("b0f", [P, 3 * (h // P)], F32, kind="ExternalInput")
    b0b = nc.dram_tensor("b0b", [P, 3 * (h // P)], F32, kind="ExternalInput")
    w1f = nc.dram_tensor("w1f", [2 * h, 3 * h], FP16, kind="ExternalInput")
    w1b = nc.dram_tensor("w1b", [2 * h, 3 * h], FP16, kind="ExternalInput")
    b1f = nc.dram_tensor("b1f", [P, 3 * (h // P)], F32, kind="ExternalInput")
    b1b = nc.dram_tensor("b1b", [P, 3 * (h // P)], F32, kind="ExternalInput")
    out_t = nc.dram_tensor("out_t", [bc, 2 * h, s], F32, kind="ExternalOutput")

    ns = s // s_tile
    hc = h // P
    k0 = d // P       # layer-0 contraction chunks
    k1 = 2 * h // P   # layer-1 contraction chunks

    with TileContext(nc) as tc:
        with (
            tc.tile_pool(name="wpool", bufs=1) as wpool,     # fp16 weights
            tc.tile_pool(name="y1pool", bufs=1) as y1pool,   # inter-layer act
            tc.tile_pool(name="scr", bufs=3) as spool,       # zn/f/o/g/c/y
            tc.tile_pool(name="carry", bufs=1) as cpool,
            tc.tile_pool(name="instream", bufs=1) as ypool,  # layer-0 input
            tc.tile_pool(name="ps", bufs=1, space="PSUM") as ppool,
        ):
            # ---------------- weights (plain fp16 DMAs) ----------------
            wk = {
                nm: [wpool.tile([P, 3 * h], FP16, tag=f"{nm}_{k}",
                                name=f"{nm}_{k}")
                     for k in range(kc)]
                for nm, kc in (("w0f", k0), ("w0b", k0),
                               ("w1f", k1), ("w1b", k1))
            }

            def load_w(name, wd, k_chunks):
                for k in range(k_chunks):
                    nc.sync.dma_start(wk[name][k][:], wd[k * P : (k + 1) * P, :])

            def load_biases(bd, prefix):
                """One plain DMA of the host-pretabled [P, 3*hc] bias;
                returns per-(gate, h-chunk) [P, 1] views."""
                btab = wpool.tile([P, 3 * hc], F32, tag=f"{prefix}_btab",
                                  name=f"{prefix}_btab")
                nc.sync.dma_start(btab[:], bd[:, :])
                return {
                    (g, hh): btab[:, g * hc + hh : g * hc + hh + 1]
                    for g in range(3)
                    for hh in range(hc)
                }

            # PE warm-up: dummy matmuls on memset scratch keep the PE
            # busy through the DMA bring-up window, so the p-state governor
            # is at full clock when the first real matmul issues.
            warm_w = wpool.tile([P, P], FP16, tag="warm_w", name="warm_w")
            warm_m = wpool.tile([P, s_tile], FP16, tag="warm_m", name="warm_m")
            nc.gpsimd.memset(warm_w[:], 0.0)
            nc.gpsimd.memset(warm_m[:], 0.0)
            warm_ps = ppool.tile([P, s_tile], F32, tag="ps2", name="warm_ps",
                                 bufs=2)
            for _ in range(16):
                nc.tensor.matmul(warm_ps[:], warm_w[:], warm_m[:],
                                 start=True, stop=True)

            bt = {}
            # Queue order at startup: the first matmul needs w0f chunk 0
            # (scalar q) + input tile 0 (sync q); everything else comes
            # after those on their queues.
            load_w("w0f", w0f, k0)

            # ---------------- layer-0 input prefetch ----------------
            l0_iters = []
            for b in range(bc):
                for fw in (True, False):
                    order = range(ns) if fw else range(ns - 1, -1, -1)
                    for si in order:
                        l0_iters.append((b, fw, si))
            l0_pos = {key: i for i, key in enumerate(l0_iters)}
            in_tiles = {}

            def issue_l0_input(key):
                b, fw, si = key
                s0 = si * s_tile
                tiles = []
                for k in range(k0):
                    t = ypool.tile([P, s_tile], FP16, tag=f"inr{k}", bufs=2,
                                   name=f"inr{k}")
                    nc.sync.dma_start(
                        t[:], xt[b, k * P : (k + 1) * P, s0 : s0 + s_tile])
                    tiles.append(t)
                in_tiles[key] = tiles

            issue_l0_input(l0_iters[0])
            # Remaining weights: queue now, they drain through slack long
            # before first use (w0b ~45us in, w1f ~90us in).
            bt["w0f"] = load_biases(b0f, "b0f")
            bt["w0b"] = load_biases(b0b, "b0b")
            bt["w1f"] = load_biases(b1f, "b1f")
            load_w("w0b", w0b, k0)
            load_w("w1f", w1f, k1)

            y1c = {}  # chunk index 0..7 -> current row's SBUF tile

            for b in range(bc):
                for layer in (0, 1):
                    for fw in ((True, False) if layer == 0 else (False, True)):
                        wname = f"w{layer}{'f' if fw else 'b'}"
                        kch = k0 if layer == 0 else k1
                        dir_off = 0 if fw else h
                        s_order = (list(range(ns)) if fw
                                   else list(range(ns - 1, -1, -1)))
                        if b == 0 and layer == 0 and not fw:
                            bt["w1b"] = load_biases(b1b, "b1b")
                            load_w("w1b", w1b, k1)
                        btile = bt[wname]
                        carry = [cpool.tile([P, 1], F32, tag=f"c{hh}",
                                            name=f"carry{hh}")
                                 for hh in range(hc)]
                        if layer == 0:
                            off = 0 if fw else hc
                            for c in range(hc):
                                y1c[off + c] = y1pool.tile(
                                    [P, s], FP16, tag=f"y1c{off + c}",
                                    name=f"y1c{off + c}")
                        for si, s_idx in enumerate(s_order):
                            s0 = s_idx * s_tile
                            if layer == 0:
                                pos = l0_pos[(b, fw, s_idx)]
                                if pos + 1 < len(l0_iters):
                                    issue_l0_input(l0_iters[pos + 1])
                                ins = [t[:] for t in
                                       in_tiles.pop((b, fw, s_idx))]
                            else:
                                ins = [y1c[k][:, s0 : s0 + s_tile]
                                       for k in range(k1)]
                            for hh in range(hc):
                                ps = [
                                    ppool.tile([P, s_tile], F32, tag=f"ps{g}",
                                               name=f"ps{g}",
                                               bufs=(3 if g < 2 else 2))
                                    for g in range(3)
                                ]
                                for g in range(3):
                                    cols = slice(g * h + hh * P,
                                                 g * h + (hh + 1) * P)
                                    for k in range(kch):
                                        nc.tensor.matmul(
                                            ps[g][:],
                                            wk[wname][k][:, cols],
                                            ins[k],
                                            start=(k == 0),
                                            stop=(k == kch - 1),
                                        )
                                zn = spool.tile([P, s_tile], F32, tag="zn",
                                                name="zn")
                                f_ = spool.tile([P, s_tile], F32, tag="f",
                                                name="f")
                                o = spool.tile([P, s_tile], F32, tag="o",
                                               name="o")
                                g_ = spool.tile([P, s_tile], F32, tag="g",
                                                name="g")
                                c_ = spool.tile([P, s_tile], F32, tag="c",
                                                name="c")
                                nc.scalar.activation(f_[:], ps[1][:],
                                                     ACT.Sigmoid,
                                                     bias=btile[1, hh][:])
                                # host negated the z-gate W/b: Tanh gives -z
                                nc.scalar.activation(zn[:], ps[0][:], ACT.Tanh,
                                                     bias=btile[0, hh][:])
                                nc.scalar.activation(o[:], ps[2][:],
                                                     ACT.Sigmoid,
                                                     bias=btile[2, hh][:])
                                # g = (f - 1) * (-z) = (1 - f) z, one DVE op
                                nc.vector.scalar_tensor_tensor(
                                    g_[:], f_[:], 1.0, zn[:],
                                    ALU.subtract, ALU.mult)
                                # c_t = f_t * c_prev + g_t (bw: reversed time)
                                if fw:
                                    sc = (c_[:], f_[:], g_[:])
                                    carry_col = slice(s_tile - 1, s_tile)
                                else:
                                    sc = (c_[:, ::-1], f_[:, ::-1], g_[:, ::-1])
                                    carry_col = slice(0, 1)
                                init = 0.0 if si == 0 else carry[hh][:]
                                nc.vector.tensor_tensor_scan(
                                    sc[0], sc[1], sc[2], init,
                                    ALU.mult, ALU.add)
                                if si < ns - 1:
                                    nc.gpsimd.tensor_copy(carry[hh][:],
                                                          c_[:, carry_col])
                                if layer == 0:
                                    chunk = y1c[(0 if fw else hc) + hh]
                                    nc.gpsimd.tensor_mul(
                                        chunk[:, s0 : s0 + s_tile],
                                        o[:], c_[:])
                                else:
                                    y = spool.tile([P, s_tile], F32, tag="y",
                                                   name="y")
                                    nc.gpsimd.tensor_mul(y[:], o[:], c_[:])
                                    row0 = dir_off + hh * P
                                    nc.sync.dma_start(
                                        out_t[b, row0 : row0 + P,
                                              s0 : s0 + s_tile], y[:])

    nc.finalize()
    return nc


_NC_CACHE = {}


def _get_nc():
    if "v3" not in _NC_CACHE:
        _NC_CACHE["v3"] = build_nc()
    return _NC_CACHE["v3"]


def kernel(X, seqlens, W_fw0, b_fw0, W_bw0, b_bw0, W_fw1, b_fw1, W_bw1, b_bw1,
           mm_dtype="fp16", trace=False):
    """Full-input entry point: shards over 8 cores, returns [B, S, 2H] f32."""
    del seqlens, mm_dtype  # seqlens unused by the reference computation
    X = np.asarray(X, dtype=np.float32)

    def neg_z(v, dt):
        """Negate the z-gate block so tanh(ps+b) = -z on device."""
        v = np.array(np.asarray(v, dtype=np.float32))
        v[..., :H] *= -1.0
        return np.ascontiguousarray(v.astype(dt))

    def btab(v):
        """[3H] bias -> [P, 3*H/P] column table (z-gate negated)."""
        v = neg_z(v, np.float32)  # [3H]
        t = v.reshape(3 * H // P, P).T  # [P, 3hc]
        return np.ascontiguousarray(t)

    weights = {
        "w0f": neg_z(W_fw0, np.float16), "b0f": btab(b_fw0),
        "w0b": neg_z(W_bw0, np.float16), "b0b": btab(b_bw0),
        "w1f": neg_z(W_fw1, np.float16), "b1f": btab(b_fw1),
        "w1b": neg_z(W_bw1, np.float16), "b1b": btab(b_bw1),
    }

    nc = _get_nc()
    in_maps = []
    for i in range(N_CORES):
        rows = X[i * BC : (i + 1) * BC]  # [BC, S, D]
        xt_i = np.ascontiguousarray(
            rows.transpose(0, 2, 1).astype(np.float16))  # [BC, D, S] fp16
        in_maps.append({"xt": xt_i, **weights})

    res = bass_utils.run_bass_kernel_spmd(
        nc, in_maps, core_ids=list(range(N_CORES)), trace=trace
    )
    out = np.empty((B, S, 2 * H), dtype=np.float32)
    for i in range(N_CORES):
        out_t = res.results[i]["out_t"]  # [BC, 2H, S]
        out[i * BC : (i + 1) * BC] = out_t.transpose(0, 2, 1)
    kernel.last_results = res
    return out
